# revision 30
# baseline (speedup 1.0000x reference)
"""GQA attention kernel for Trainium2, 8 NeuronCores — v4.

Problem: B=1, S=4096, HIDDEN=2048, 8 query heads x d=256, 1 shared KV head,
causal mask, fp32 in/out.

Sharding: full tensor-parallel over heads with a replicated input.
Host-side, x is transposed/replicated to all cores in bf16 with an SBUF-
matched layout so every big load is ONE coalesced DMA.
Core j owns head j end-to-end:
  1. project own 512-key kv slice; AllGather kvT (d-major) AND a PE-
     transposed rows-major copy (second AllGather) so no per-core
     32x2 transpose sweep is needed. Collective input DMAs go through
     the gpsimd SWDGE queue so they don't queue behind the fat xT
     streams on the HWDGE FIFOs.
  2. stream xT in 4 chunks of 1024 rows; project q (own head only).
  3. causal flash attention for head j over all 4096 rows (fp32 PSUM, no
     max subtraction), diagonal blocks narrowed to the causal range.
     Denominator via ones-column matmuls (same PE cost as the fp8
     DoubleRow trick, but no duplicate fp8 exp stream on scalar).
     Exps fused to [128,1024] ACTIVATEs via 2-bank PSUM score tiles.
  4. normalize, one bf16 AllToAll (head-major -> row-block-major).
  5. output projection of own 512-row block against full wo.
Host concatenates the 8 row blocks.
"""

import sys

import numpy as np

sys.path.insert(0, "/opt/trn_rl_repo")

S = 4096
HID = 2048
NH = 8
D = 256
NCORES = 8
R = 512  # output rows per core / q rows per attention block
CH = 1024  # projection chunk rows
NCH = S // CH
NEG = -1.0e9
SCALE = 1.0 / 16.0  # 1/sqrt(256)

_BUILT = None


def _build():
    global _BUILT
    if _BUILT is not None:
        return _BUILT

    from contextlib import ExitStack

    import ml_dtypes

    from concourse import bacc, tile
    from concourse.bass import mybir

    dt = mybir.dt
    f32 = dt.float32
    bf16 = dt.bfloat16
    bfnp = ml_dtypes.bfloat16
    AF = mybir.ActivationFunctionType

    nc = bacc.Bacc(
        "TRN2",
        target_bir_lowering=False,
        debug=False,
        num_devices=NCORES,
    )

    # ---- DRAM I/O (host-side layouts matched to SBUF tiles) ----
    # xT_d[c, p, hs*CH+col] = x[CH*c+col, 128*hs+p]
    xT_d = nc.dram_tensor("xT", [NCH, 128, 16 * CH], bf16, kind="ExternalInput")
    # wqkv_d[p, hs*512 + j] = (wq_head | wkv)[128*hs+p, j]  (j<256 -> wq)
    wqkv_d = nc.dram_tensor("wqkv", [128, 16 * 512], bf16, kind="ExternalInput")
    # bqkv_d columns: [bq_dh0, bq_dh1, bkv_dh0, bkv_dh1]
    bqkv_d = nc.dram_tensor("bqkv", [128, 4], f32, kind="ExternalInput")
    # wo_d[p, k*HID + col] = wo2[(128*k+p), col]
    wo_d = nc.dram_tensor("wo2d", [128, 16 * HID], bf16, kind="ExternalInput")
    bo_row = nc.dram_tensor("bo_row", [1, HID], bf16, kind="ExternalInput")
    out = nc.dram_tensor("out", [R, HID], bf16, kind="ExternalOutput")

    # kvx_d[p, hs*512+col] = x[512*j+col, 128*hs+p] for core j (own key slice)
    kvx_d = nc.dram_tensor("kvx", [128, 16 * 512], bf16, kind="ExternalInput")

    # ---- collective buffers ----
    grp = [list(range(NCORES))]
    ao_send = nc.dram_tensor("ao_send", [NH * D, R], bf16)
    ao_recv = nc.dram_tensor("ao_recv", [NH * D, R], bf16)
    # kv AllGather: cols 0:512 kvT dh0, 512:1024 kvT dh1 (d-major only;
    # rows-major kv is rebuilt locally by PE transposes in the phase-1
    # tail where the PE idles on the collective anyway)
    kvm_send = nc.dram_tensor("kvm_send", [128, 1024], bf16)
    kvm_all = nc.dram_tensor(
        "kvm_all", [NCORES * 128, 1024], bf16, addr_space="Shared"
    )
    # tiny collective issued at t=0 to absorb the one-time ncfw bootstrap
    warm_send = nc.dram_tensor("warm_send", [128, 4], bf16)
    warm_all = nc.dram_tensor(
        "warm_all", [NCORES * 128, 4], bf16, addr_space="Shared"
    )

    # ---- compile-time constants ----
    ident_np = np.eye(128).astype(bfnp)
    ones_col_np = np.ones((128, 1)).astype(bfnp)
    # one shared diagonal mask: every diagonal (grel, sl) sub-block equals
    # m0[kappa, col - rel0] with m0[kappa, c] = NEG iff kappa > c
    kappa = np.arange(128)[:, None]
    cols = np.arange(512)[None, :]
    mask_np = np.where(kappa <= cols, 0.0, NEG).astype(np.float32)
    ident_d = nc.inline_tensor(ident_np, "ident")
    ones_col_d = nc.inline_tensor(ones_col_np, "ones_col")
    mask_d = nc.inline_tensor(mask_np, "mask_const")

    with tile.TileContext(nc) as tc:
        with ExitStack() as top:
            # fire the bootstrap-absorbing dummy collective immediately
            nc.gpsimd.collective_compute(
                "AllGather",
                mybir.AluOpType.bypass,
                replica_groups=grp,
                ins=[warm_send[:]],
                outs=[warm_all[:]],
            )
            cpool = top.enter_context(tc.tile_pool(name="const", bufs=1))

            # pools alive through projections + attention; phase 3 fits
            # alongside (aoT/osb ~36KB + big 50KB + wo 64KB < 208KB)
            big = top.enter_context(tc.tile_pool(name="big", bufs=1))
            qT = big.tile([128, 2 * S], bf16, tag="qT")  # [d-slice, rows]
            kvT = big.tile([128, 2 * S], bf16, tag="kvT")  # [d-slice, keys]
            kv_sb = big.tile([128, 32 * D], bf16, tag="kv")  # rows-major kv
            mask_sb = big.tile([128, 512], f32, tag="mask")

            with ExitStack() as ph1:
                wpool = ph1.enter_context(tc.tile_pool(name="w", bufs=1))
                xr_pool = ph1.enter_context(tc.tile_pool(name="xr", bufs=6))

                # critical-path loads first, split in half-chunks so the
                # first projection matmuls can start ~3us earlier
                wqkv_sb = wpool.tile([128, 16 * 512], bf16, tag="wqkv")
                kvx_sb = wpool.tile([128, 16 * 512], bf16, tag="kvx")
                for hh in range(2):
                    sl = slice(4096 * hh, 4096 * hh + 4096)
                    nc.sync.dma_start(wqkv_sb[:, sl], wqkv_d[:, sl])
                    nc.scalar.dma_start(kvx_sb[:, sl], kvx_d[:, sl])

                HCH = 8 * CH  # half-chunk: hs 0-7 / 8-15
                xrh = {}
                for c, hh in ((0, 0), (0, 1), (1, 0), (1, 1)):
                    t = xr_pool.tile([128, HCH], bf16, tag="xr", name=f"xr{c}_{hh}")
                    eng = nc.scalar if hh == 0 else nc.sync
                    eng.dma_start(t[:], xT_d[c, :, HCH * hh : HCH * hh + HCH])
                    xrh[(c, hh)] = t

                # remaining constants
                ident = cpool.tile([128, 128], bf16, tag="ident")
                nc.sync.dma_start(ident[:], ident_d[:])
                ones_col = cpool.tile([128, 1], bf16, tag="ones_col")
                nc.sync.dma_start(ones_col[:], ones_col_d[:])
                bqkv_sb = cpool.tile([128, 4], f32, tag="bqkv")
                nc.sync.dma_start(bqkv_sb[:], bqkv_d[:])
                nc.sync.dma_start(mask_sb[:], mask_d[:])

                pj_psum = ph1.enter_context(
                    tc.tile_pool(name="pj_psum", bufs=6, space="PSUM")
                )
                tp_psum = ph1.enter_context(
                    tc.tile_pool(name="tp_psum", bufs=2, space="PSUM")
                )

                # ---- own 512-key kv projection (d-major) ----
                # hs-outer loop so matmuls start as soon as the first half
                # of the weights lands; two interleaved PSUM groups
                kvs_pool = ph1.enter_context(tc.tile_pool(name="kvs", bufs=2))
                kvs = {}
                kps = [
                    pj_psum.tile([128, 512], f32, tag="pj", name=f"kvp{dh}")
                    for dh in range(2)
                ]
                for hs in range(16):
                    for dh in range(2):
                        nc.tensor.matmul(
                            kps[dh][:],
                            wqkv_sb[
                                :,
                                512 * hs + 256 + 128 * dh : 512 * hs
                                + 256
                                + 128 * dh
                                + 128,
                            ],
                            kvx_sb[:, 512 * hs : 512 * hs + 512],
                            start=(hs == 0),
                            stop=(hs == 15),
                        )
                for dh in range(2):
                    t = kvs_pool.tile([128, 512], bf16, tag="kvs", name=f"kvs{dh}")
                    nc.vector.tensor_scalar_add(
                        t[:], kps[dh][:], bqkv_sb[:, 2 + dh : 3 + dh]
                    )
                    # SWDGE queue: doesn't sit behind the fat xT loads
                    nc.gpsimd.dma_start(
                        kvm_send[:, 512 * dh : 512 * dh + 512], t[:]
                    )
                    kvs[dh] = t

                nc.gpsimd.collective_compute(
                    "AllGather",
                    mybir.AluOpType.bypass,
                    replica_groups=grp,
                    ins=[kvm_send[:]],
                    outs=[kvm_all[:]],
                )



                # ---- q projection (own head) over streamed half-chunks
                for c in range(NCH):
                    late = (
                        [(c + 2, 0), (c + 2, 1)] if c + 2 < NCH else []
                    )
                    for lc, hh in late:
                        t = xr_pool.tile(
                            [128, HCH], bf16, tag="xr", name=f"xr{lc}_{hh}"
                        )
                        eng = nc.scalar if hh == 0 else nc.sync
                        eng.dma_start(
                            t[:], xT_d[lc, :, HCH * hh : HCH * hh + HCH]
                        )
                        xrh[(lc, hh)] = t
                    if c == NCH - 1:
                        # post-AllGather kvT pulls on both fast HWDGE queues
                        for src_c in range(NCORES):
                            for dh in range(2):
                                eng = nc.sync if dh == 0 else nc.scalar
                                eng.dma_start(
                                    kvT[
                                        :,
                                        S * dh + R * src_c : S * dh + R * src_c + R,
                                    ],
                                    kvm_all[
                                        128 * src_c : 128 * src_c + 128,
                                        512 * dh : 512 * dh + 512,
                                    ],
                                )
                    for dh in range(2):
                        for half in range(CH // 512):
                            pp = pj_psum.tile([128, 512], f32, tag="pj")
                            for hs in range(16):
                                xt = xrh[(c, hs // 8)]
                                nc.tensor.matmul(
                                    pp[:],
                                    wqkv_sb[
                                        :,
                                        512 * hs + 128 * dh : 512 * hs + 128 * dh + 128,
                                    ],
                                    xt[
                                        :,
                                        CH * (hs % 8) + 512 * half : CH * (hs % 8)
                                        + 512 * half
                                        + 512,
                                    ],
                                    start=(hs == 0),
                                    stop=(hs == 15),
                                )
                            nc.vector.tensor_scalar_add(
                                qT[
                                    :,
                                    S * dh + CH * c + 512 * half : S * dh
                                    + CH * c
                                    + 512 * half
                                    + 512,
                                ],
                                pp[:],
                                bqkv_sb[:, dh : dh + 1],
                            )

                # rebuild rows-major kv locally: 64 PE transposes in the
                # phase-1 tail (PE is gated on the AllGather here anyway)
                for src_c in range(NCORES):
                    for t4 in range(4):
                        for dh in range(2):
                            kt = 4 * src_c + t4
                            tp = tp_psum.tile([128, 128], bf16, tag="tp")
                            nc.tensor.transpose(
                                tp[:],
                                kvT[
                                    :,
                                    S * dh + 128 * kt : S * dh + 128 * kt + 128,
                                ],
                                ident[:],
                            )
                            nc.vector.tensor_copy(
                                kv_sb[:, D * kt + 128 * dh : D * kt + 128 * dh + 128],
                                tp[:],
                            )

            # wo pool entered only now so phase 1 can use the space for
            # deeper xT buffering; its DMA is issued inside the attention
            # loop (after block 0)
            wo_pool = top.enter_context(tc.tile_pool(name="wo", bufs=1))
            wo_sb = wo_pool.tile([128, 16 * HID], bf16, tag="wo")

            # ============ phase 2: causal flash attention ============
            with ExitStack() as ph2:
                s_psum = ph2.enter_context(
                    tc.tile_pool(name="s_psum", bufs=2, space="PSUM")
                )
                ao_psum = ph2.enter_context(
                    tc.tile_pool(name="ao_psum", bufs=3, space="PSUM")
                )
                den_psum = ph2.enter_context(
                    tc.tile_pool(name="den_psum", bufs=1, space="PSUM")
                )
                p_pool = ph2.enter_context(tc.tile_pool(name="p", bufs=3))
                nrm_pool = ph2.enter_context(tc.tile_pool(name="nrm", bufs=2))
                aon_pool = ph2.enter_context(tc.tile_pool(name="aon", bufs=4))

                for b in range(8):
                    aops = [
                        ao_psum.tile([128, R], f32, tag="aops", name=f"aops{b}_{i}")
                        for i in range(2)
                    ]
                    denp = den_psum.tile([1, R], f32, tag="denp")
                    ngroups = 2 * (b + 1)
                    nkeys = 2 * ngroups  # 128-key slices in this block

                    def consume(item):
                        k, r0, pt_c = item
                        sl = k % 2
                        first = k == 0
                        last = k == nkeys - 1
                        # AV first so a denominator-bank wait can't block
                        # the AV matmuls behind it in the PE FIFO
                        for dh in range(2):
                            nc.tensor.matmul(
                                aops[dh][:, r0:R],
                                kv_sb[:, D * k + 128 * dh : D * k + 128 * dh + 128],
                                pt_c[:, 512 * sl + r0 : 512 * sl + 512],
                                start=first,
                                stop=last,
                            )
                        nc.tensor.matmul(
                            denp[0:1, r0:R],
                            ones_col[:],
                            pt_c[:, 512 * sl + r0 : 512 * sl + 512],
                            start=first,
                            stop=last,
                            skip_group_check=True,
                        )

                    pending = []

                    def consume_all():
                        for it in pending:
                            consume(it)
                        del pending[:]

                    for kg in range(ngroups):
                        diag = kg >= 2 * b
                        grel = kg - 2 * b
                        st = s_psum.tile([128, 1024], f32, tag="st")
                        pt = p_pool.tile([128, 1024], bf16, tag="pt")
                        if not diag:
                            for sl in range(2):
                                k = 2 * kg + sl
                                for dh in range(2):
                                    nc.tensor.matmul(
                                        st[:, 512 * sl : 512 * sl + 512],
                                        kvT[
                                            :,
                                            S * dh + 128 * k : S * dh + 128 * k + 128,
                                        ],
                                        qT[:, S * dh + R * b : S * dh + R * b + R],
                                        start=(dh == 0),
                                        stop=(dh == 1),
                                    )
                            # one fused ACTIVATE over both key slices
                            nc.scalar.activation(
                                pt[:, 0:1024], st[:, 0:1024], AF.Exp, scale=SCALE
                            )
                            consume_all()
                            pending.append((2 * kg, 0, pt))
                            pending.append((2 * kg + 1, 0, pt))
                        else:
                            for sl in range(2):
                                r0 = 256 * grel + 128 * sl
                                k = 2 * kg + sl
                                for dh in range(2):
                                    nc.tensor.matmul(
                                        st[:, 512 * sl + r0 : 512 * sl + 512],
                                        kvT[
                                            :,
                                            S * dh + 128 * k : S * dh + 128 * k + 128,
                                        ],
                                        qT[
                                            :,
                                            S * dh + R * b + r0 : S * dh + R * b + R,
                                        ],
                                        start=(dh == 0),
                                        stop=(dh == 1),
                                    )
                                nc.vector.tensor_add(
                                    st[:, 512 * sl + r0 : 512 * sl + 512],
                                    st[:, 512 * sl + r0 : 512 * sl + 512],
                                    mask_sb[:, 0 : 512 - r0],
                                )
                                nc.scalar.activation(
                                    pt[:, 512 * sl + r0 : 512 * sl + 512],
                                    st[:, 512 * sl + r0 : 512 * sl + 512],
                                    AF.Exp,
                                    scale=SCALE,
                                )
                                consume_all()
                                pending.append((k, r0, pt))
                    consume_all()
                    # evacuate PSUM fast (frees aops for the next block's
                    # AV matmuls), then normalize in SBUF at 2x DVE rate
                    raws = []
                    for dh in range(2):
                        raw = aon_pool.tile([128, R], bf16, tag="raw")
                        nc.vector.tensor_copy(raw[:], aops[dh][:])
                        raws.append(raw)
                    den_sb = nrm_pool.tile([1, R], bf16, tag="den_sb")
                    with nc.allow_low_precision(
                        reason="1/den in bf16: |rel err| ~0.4% vs 2e-2 budget"
                    ):
                        nc.vector.reciprocal(den_sb[:], denp[0:1, :])
                    bc = nrm_pool.tile([128, R], bf16, tag="bc")
                    nc.gpsimd.partition_broadcast(bc[:], den_sb[:])
                    for dh in range(2):
                        aon = aon_pool.tile([128, R], bf16, tag="aon")
                        nc.vector.tensor_mul(aon[:], raws[dh][:], bc[:])
                        nc.sync.dma_start(
                            ao_send[D * b + 128 * dh : D * b + 128 * dh + 128, :],
                            aon[:],
                        )
                    if b == 3:
                        # prefetch wo only now: keeps the scalar DMA queue
                        # free for the kv/xT tail during the transition
                        nc.scalar.dma_start(wo_sb[:], wo_d[:])
                nc.gpsimd.collective_compute(
                    "AllToAll",
                    mybir.AluOpType.bypass,
                    replica_groups=grp,
                    ins=[ao_send[:]],
                    outs=[ao_recv[:]],
                )

            # ============ phase 3: output projection ============
            with ExitStack() as ph3:
                o_in = ph3.enter_context(tc.tile_pool(name="o_in", bufs=1))
                bor_sb = o_in.tile([1, HID], bf16, tag="bor")
                nc.sync.dma_start(bor_sb[:], bo_row[:])
                bo_bc = o_in.tile([128, HID], bf16, tag="bo_bc")
                nc.gpsimd.partition_broadcast(bo_bc[:], bor_sb[:])
                # 4 aoT tiles so the first out-proj matmuls start after the
                # first quarter of the loads lands
                aoTs = []
                for g in range(4):
                    t = o_in.tile([128, 4 * R], bf16, tag=f"aoT{g}")
                    for kk in range(4):
                        k = 4 * g + kk
                        eng = nc.sync if k % 2 == 0 else nc.scalar
                        eng.dma_start(
                            t[:, R * kk : R * kk + R],
                            ao_recv[128 * k : 128 * k + 128, :],
                        )
                    aoTs.append(t)
                o_psum = ph3.enter_context(
                    tc.tile_pool(name="o_psum", bufs=4, space="PSUM")
                )
                o_out = ph3.enter_context(tc.tile_pool(name="o_out", bufs=2))
                for rc in range(4):
                    osb = o_out.tile([128, HID], bf16, tag="osb")
                    for ncol in range(4):
                        ps = o_psum.tile([128, 512], f32, tag="ops")
                        for k in range(16):
                            nc.tensor.matmul(
                                ps[:],
                                aoTs[k // 4][:, R * (k % 4) + 128 * rc : R * (k % 4) + 128 * rc + 128],
                                wo_sb[
                                    :,
                                    HID * k + 512 * ncol : HID * k + 512 * ncol + 512,
                                ],
                                start=(k == 0),
                                stop=(k == 15),
                            )
                        with nc.allow_low_precision(
                            reason="bf16 output rounding ~0.4% vs 2e-2 budget"
                        ):
                            nc.vector.tensor_add(
                                osb[:, 512 * ncol : 512 * ncol + 512],
                                ps[:],
                                bo_bc[:, 512 * ncol : 512 * ncol + 512],
                            )
                    eng = nc.sync if rc % 2 == 0 else nc.scalar
                    eng.dma_start(out[128 * rc : 128 * rc + 128, :], osb[:])

    nc.compile()
    _BUILT = nc
    return nc


def _make_in_maps(x, wq, bq, wkv, bkv, wo, bo):
    import ml_dtypes

    bfnp = ml_dtypes.bfloat16
    x2d = np.asarray(x, dtype=np.float32).reshape(S, HID)
    # xT_d[c, p, hs*CH+col] = x[CH*c+col, 128*hs+p]
    xT = (
        x2d.reshape(NCH, CH, 16, 128)
        .transpose(0, 3, 2, 1)
        .reshape(NCH, 128, 16 * CH)
        .astype(bfnp)
    )
    wq3 = np.asarray(wq, dtype=np.float32).reshape(HID, NH, D)
    bq2 = np.asarray(bq, dtype=np.float32).reshape(NH, D)
    bkv1 = np.asarray(bkv, dtype=np.float32).reshape(D)
    wkv2 = np.asarray(wkv, dtype=np.float32).reshape(HID, D)
    wo2 = np.asarray(wo, dtype=np.float32).reshape(HID, HID)
    wo_h = (
        wo2.reshape(16, 128, HID).transpose(1, 0, 2).reshape(128, 16 * HID).astype(bfnp)
    )
    shared = {
        "xT": xT,
        "wo2d": wo_h,
        "bo_row": np.asarray(bo, dtype=np.float32).reshape(1, HID).astype(bfnp),
    }
    # kvx[p, hs*512+col] = x[512*j+col, 128*hs+p]
    xr4 = x2d.reshape(NCORES, 512, 16, 128)  # [j, col, hs, p]
    in_maps = []
    for j in range(NCORES):
        m = dict(shared)
        m["kvx"] = (
            np.ascontiguousarray(xr4[j].transpose(2, 1, 0))
            .reshape(128, 16 * 512)
            .astype(bfnp)
        )
        wq_h = wq3[:, j, :]  # [HID, D]
        qk = np.concatenate(
            [wq_h.reshape(16, 128, D), wkv2.reshape(16, 128, D)], axis=2
        )  # [16, 128, 512]
        m["wqkv"] = qk.transpose(1, 0, 2).reshape(128, 16 * 512).astype(bfnp)
        bq_h = bq2[j]
        bqkv = np.stack(
            [bq_h[:128], bq_h[128:], bkv1[:128], bkv1[128:]], axis=1
        )  # [128, 4]
        m["bqkv"] = np.ascontiguousarray(bqkv.astype(np.float32))
        in_maps.append(m)
    return in_maps


def _run(inputs, trace=False, **trace_kwargs):
    from concourse.bass_utils import run_bass_kernel_spmd

    nc = _build()
    in_maps = _make_in_maps(
        inputs["x"],
        inputs["wq"],
        inputs["bq"],
        inputs["wkv"],
        inputs["bkv"],
        inputs["wo"],
        inputs["bo"],
    )
    res = run_bass_kernel_spmd(
        nc, in_maps, list(range(NCORES)), trace=trace, **trace_kwargs
    )
    outs = [np.asarray(res.results[j]["out"]) for j in range(NCORES)]
    full = np.concatenate(outs, axis=0).reshape(1, S, HID).astype(np.float32)
    return full, res


def kernel(**inputs):
    full, _ = _run(inputs, trace=False)
    return full


if __name__ == "__main__":
    rng = np.random.default_rng(0)
    ins = {
        "x": rng.standard_normal((1, S, HID), dtype=np.float32),
        "wq": rng.standard_normal((HID, NH, D), dtype=np.float32) / 45.25,
        "bq": np.zeros((NH, D), np.float32),
        "wkv": rng.standard_normal((HID, 1, D), dtype=np.float32) / 45.25,
        "bkv": np.zeros((1, D), np.float32),
        "wo": rng.standard_normal((NH, D, HID), dtype=np.float32) / 45.25,
        "bo": np.zeros((HID,), np.float32),
        "mask": np.tril(np.ones((S, S), bool))[None, None],
    }
    out = kernel(**ins)
    print("out", out.shape, out.dtype, float(np.abs(out).max()))


# revision 34
# speedup vs baseline: 1.0717x; 1.0717x over previous
"""GQA attention kernel for Trainium2, 8 NeuronCores — v4.

Problem: B=1, S=4096, HIDDEN=2048, 8 query heads x d=256, 1 shared KV head,
causal mask, fp32 in/out.

Sharding: full tensor-parallel over heads with a replicated input.
Host-side, x is transposed/replicated to all cores in bf16 with an SBUF-
matched layout so every big load is ONE coalesced DMA.
Core j owns head j end-to-end:
  1. project own 512-key kv slice; AllGather kvT (d-major) AND a PE-
     transposed rows-major copy (second AllGather) so no per-core
     32x2 transpose sweep is needed. Collective input DMAs go through
     the gpsimd SWDGE queue so they don't queue behind the fat xT
     streams on the HWDGE FIFOs.
  2. stream xT in 4 chunks of 1024 rows; project q (own head only).
  3. causal flash attention for head j over all 4096 rows (fp32 PSUM, no
     max subtraction), diagonal blocks narrowed to the causal range.
     Denominator via ones-column matmuls (same PE cost as the fp8
     DoubleRow trick, but no duplicate fp8 exp stream on scalar).
     Exps fused to [128,1024] ACTIVATEs via 2-bank PSUM score tiles.
  4. normalize, one bf16 AllToAll (head-major -> row-block-major).
  5. output projection of own 512-row block against full wo.
Host concatenates the 8 row blocks.
"""

import sys

import numpy as np

sys.path.insert(0, "/opt/trn_rl_repo")

S = 4096
HID = 2048
NH = 8
D = 256
NCORES = 8
R = 512  # output rows per core / q rows per attention block
CH = 1024  # projection chunk rows
NCH = S // CH
NEG = -1.0e9
SCALE = 1.0 / 16.0  # 1/sqrt(256)

_BUILT = None


def _build():
    global _BUILT
    if _BUILT is not None:
        return _BUILT

    from contextlib import ExitStack

    import ml_dtypes

    from concourse import bacc, tile
    from concourse.bass import mybir

    dt = mybir.dt
    f32 = dt.float32
    bf16 = dt.bfloat16
    bfnp = ml_dtypes.bfloat16
    AF = mybir.ActivationFunctionType

    nc = bacc.Bacc(
        "TRN2",
        target_bir_lowering=False,
        debug=False,
        num_devices=NCORES,
    )

    # ---- DRAM I/O (host-side layouts matched to SBUF tiles) ----
    # xT_d[c, p, hs*CH+col] = x[CH*c+col, 128*hs+p]
    xT_d = nc.dram_tensor("xT", [NCH, 128, 16 * CH], bf16, kind="ExternalInput")
    # wqkv_d[p, hs*512 + j] = (wq_head | wkv)[128*hs+p, j]  (j<256 -> wq)
    wqkv_d = nc.dram_tensor("wqkv", [128, 16 * 512], bf16, kind="ExternalInput")
    # bqkv_d columns: [bq_dh0, bq_dh1, bkv_dh0, bkv_dh1]
    bqkv_d = nc.dram_tensor("bqkv", [128, 4], f32, kind="ExternalInput")
    # wo_d[p, k*HID + col] = wo2[(128*k+p), col]
    wo_d = nc.dram_tensor("wo2d", [128, 16 * HID], bf16, kind="ExternalInput")
    bo_row = nc.dram_tensor("bo_row", [1, HID], bf16, kind="ExternalInput")
    out = nc.dram_tensor("out", [R, HID], bf16, kind="ExternalOutput")

    # kvx_d[p, hs*512+col] = x[512*j+col, 128*hs+p] for core j (own key slice)
    kvx_d = nc.dram_tensor("kvx", [128, 16 * 512], bf16, kind="ExternalInput")

    # ---- collective buffers ----
    grp = [list(range(NCORES))]
    ao_send = nc.dram_tensor("ao_send", [NH * D, R], bf16)
    ao_recv = nc.dram_tensor("ao_recv", [NH * D, R], bf16)
    # one merged kv AllGather: cols 0:512 kvT dh0, 512:1024 kvT dh1,
    # 1024:2048 rows-major kv (krows[p, 256*t + d] = kv[512*j+128*t+p, d])
    kvm_send = nc.dram_tensor("kvm_send", [128, 2048], bf16)
    kvm_all = nc.dram_tensor(
        "kvm_all", [NCORES * 128, 2048], bf16, addr_space="Shared"
    )
    # tiny collective issued at t=0 to absorb the one-time ncfw bootstrap
    warm_send = nc.dram_tensor("warm_send", [128, 4], bf16)
    warm_all = nc.dram_tensor(
        "warm_all", [NCORES * 128, 4], bf16, addr_space="Shared"
    )

    # ---- compile-time constants ----
    ident_np = np.eye(128).astype(bfnp)
    ones_col_np = np.ones((128, 1)).astype(bfnp)
    # one shared diagonal mask: every diagonal (grel, sl) sub-block equals
    # m0[kappa, col - rel0] with m0[kappa, c] = NEG iff kappa > c
    kappa = np.arange(128)[:, None]
    cols = np.arange(512)[None, :]
    mask_np = np.where(kappa <= cols, 0.0, NEG).astype(np.float32)
    ident_d = nc.inline_tensor(ident_np, "ident")
    ones_col_d = nc.inline_tensor(ones_col_np, "ones_col")
    mask_d = nc.inline_tensor(mask_np, "mask_const")

    with tile.TileContext(nc) as tc:
        with ExitStack() as top:
            # fire the bootstrap-absorbing dummy collective immediately
            nc.gpsimd.collective_compute(
                "AllGather",
                mybir.AluOpType.bypass,
                replica_groups=grp,
                ins=[warm_send[:]],
                outs=[warm_all[:]],
            )
            cpool = top.enter_context(tc.tile_pool(name="const", bufs=1))

            # pools alive through projections + attention; phase 3 fits
            # alongside (aoT/osb ~36KB + big 50KB + wo 64KB < 208KB)
            big = top.enter_context(tc.tile_pool(name="big", bufs=1))
            qT = big.tile([128, 2 * S], bf16, tag="qT")  # [d-slice, rows]
            kvT = big.tile([128, 2 * S], bf16, tag="kvT")  # [d-slice, keys]
            kv_sb = big.tile([128, 32 * D], bf16, tag="kv")  # rows-major kv
            mask_sb = big.tile([128, 512], f32, tag="mask")

            with ExitStack() as ph1:
                wpool = ph1.enter_context(tc.tile_pool(name="w", bufs=1))
                xr_pool = ph1.enter_context(tc.tile_pool(name="xr", bufs=6))

                # critical-path loads first, split in half-chunks so the
                # first projection matmuls can start ~3us earlier
                wqkv_sb = wpool.tile([128, 16 * 512], bf16, tag="wqkv")
                kvx_sb = wpool.tile([128, 16 * 512], bf16, tag="kvx")
                for hh in range(2):
                    sl = slice(4096 * hh, 4096 * hh + 4096)
                    nc.sync.dma_start(wqkv_sb[:, sl], wqkv_d[:, sl])
                    nc.scalar.dma_start(kvx_sb[:, sl], kvx_d[:, sl])

                HCH = 8 * CH  # half-chunk: hs 0-7 / 8-15
                xrh = {}
                for c, hh in ((0, 0), (0, 1), (1, 0), (1, 1)):
                    t = xr_pool.tile([128, HCH], bf16, tag="xr", name=f"xr{c}_{hh}")
                    eng = nc.scalar if hh == 0 else nc.sync
                    eng.dma_start(t[:], xT_d[c, :, HCH * hh : HCH * hh + HCH])
                    xrh[(c, hh)] = t

                # remaining constants
                ident = cpool.tile([128, 128], bf16, tag="ident")
                nc.sync.dma_start(ident[:], ident_d[:])
                ones_col = cpool.tile([128, 1], bf16, tag="ones_col")
                nc.sync.dma_start(ones_col[:], ones_col_d[:])
                bqkv_sb = cpool.tile([128, 4], f32, tag="bqkv")
                nc.sync.dma_start(bqkv_sb[:], bqkv_d[:])
                nc.sync.dma_start(mask_sb[:], mask_d[:])

                pj_psum = ph1.enter_context(
                    tc.tile_pool(name="pj_psum", bufs=6, space="PSUM")
                )
                tp_psum = ph1.enter_context(
                    tc.tile_pool(name="tp_psum", bufs=2, space="PSUM")
                )

                # ---- own 512-key kv projection (d-major) ----
                # hs-outer loop so matmuls start as soon as the first half
                # of the weights lands; two interleaved PSUM groups
                kvs_pool = ph1.enter_context(tc.tile_pool(name="kvs", bufs=2))
                kvs = {}
                kps = [
                    pj_psum.tile([128, 512], f32, tag="pj", name=f"kvp{dh}")
                    for dh in range(2)
                ]
                for hs in range(16):
                    for dh in range(2):
                        nc.tensor.matmul(
                            kps[dh][:],
                            wqkv_sb[
                                :,
                                512 * hs + 256 + 128 * dh : 512 * hs
                                + 256
                                + 128 * dh
                                + 128,
                            ],
                            kvx_sb[:, 512 * hs : 512 * hs + 512],
                            start=(hs == 0),
                            stop=(hs == 15),
                        )
                for dh in range(2):
                    t = kvs_pool.tile([128, 512], bf16, tag="kvs", name=f"kvs{dh}")
                    nc.vector.tensor_scalar_add(
                        t[:], kps[dh][:], bqkv_sb[:, 2 + dh : 3 + dh]
                    )
                    # SWDGE queue: doesn't sit behind the fat xT loads
                    nc.gpsimd.dma_start(
                        kvm_send[:, 512 * dh : 512 * dh + 512], t[:]
                    )
                    kvs[dh] = t

                # rows-major copy of own keys via 8 PE transposes
                krows_pool = ph1.enter_context(tc.tile_pool(name="krows", bufs=1))
                krows_sb = krows_pool.tile([128, 1024], bf16, tag="krows")
                for t4 in range(4):
                    for dh in range(2):
                        tp = tp_psum.tile([128, 128], bf16, tag="tp")
                        nc.tensor.transpose(
                            tp[:],
                            kvs[dh][:, 128 * t4 : 128 * t4 + 128],
                            ident[:],
                        )
                        nc.vector.tensor_copy(
                            krows_sb[
                                :, 256 * t4 + 128 * dh : 256 * t4 + 128 * dh + 128
                            ],
                            tp[:],
                        )
                nc.gpsimd.dma_start(kvm_send[:, 1024:2048], krows_sb[:])
                nc.gpsimd.collective_compute(
                    "AllGather",
                    mybir.AluOpType.bypass,
                    replica_groups=grp,
                    ins=[kvm_send[:]],
                    outs=[kvm_all[:]],
                )



                # ---- q projection (own head) over streamed half-chunks
                for c in range(NCH):
                    late = (
                        [(c + 2, 0), (c + 2, 1)] if c + 2 < NCH else []
                    )
                    for lc, hh in late:
                        t = xr_pool.tile(
                            [128, HCH], bf16, tag="xr", name=f"xr{lc}_{hh}"
                        )
                        eng = nc.scalar if hh == 0 else nc.sync
                        eng.dma_start(
                            t[:], xT_d[lc, :, HCH * hh : HCH * hh + HCH]
                        )
                        xrh[(lc, hh)] = t
                    if c == NCH - 1:
                        # post-AllGather pulls on both fast HWDGE queues:
                        # kvT first (scores), then rows-major kv (AV)
                        for src_c in range(NCORES):
                            for dh in range(2):
                                eng = nc.sync if dh == 0 else nc.scalar
                                eng.dma_start(
                                    kvT[
                                        :,
                                        S * dh + R * src_c : S * dh + R * src_c + R,
                                    ],
                                    kvm_all[
                                        128 * src_c : 128 * src_c + 128,
                                        512 * dh : 512 * dh + 512,
                                    ],
                                )
                        for src_c in range(NCORES):
                            eng = nc.sync if src_c % 2 == 0 else nc.scalar
                            eng.dma_start(
                                kv_sb[:, 1024 * src_c : 1024 * src_c + 1024],
                                kvm_all[128 * src_c : 128 * src_c + 128, 1024:2048],
                            )
                    for dh in range(2):
                        for half in range(CH // 512):
                            pp = pj_psum.tile([128, 512], f32, tag="pj")
                            for hs in range(16):
                                xt = xrh[(c, hs // 8)]
                                nc.tensor.matmul(
                                    pp[:],
                                    wqkv_sb[
                                        :,
                                        512 * hs + 128 * dh : 512 * hs + 128 * dh + 128,
                                    ],
                                    xt[
                                        :,
                                        CH * (hs % 8) + 512 * half : CH * (hs % 8)
                                        + 512 * half
                                        + 512,
                                    ],
                                    start=(hs == 0),
                                    stop=(hs == 15),
                                )
                            nc.vector.tensor_scalar_add(
                                qT[
                                    :,
                                    S * dh + CH * c + 512 * half : S * dh
                                    + CH * c
                                    + 512 * half
                                    + 512,
                                ],
                                pp[:],
                                bqkv_sb[:, dh : dh + 1],
                            )

            # wo pool entered only now so phase 1 can use the space for
            # deeper xT buffering; its DMA is issued inside the attention
            # loop (after block 0)
            wo_pool = top.enter_context(tc.tile_pool(name="wo", bufs=1))
            wo_sb = wo_pool.tile([128, 16 * HID], bf16, tag="wo")

            # ============ phase 2: causal flash attention ============
            with ExitStack() as ph2:
                s_psum = ph2.enter_context(
                    tc.tile_pool(name="s_psum", bufs=2, space="PSUM")
                )
                ao_psum = ph2.enter_context(
                    tc.tile_pool(name="ao_psum", bufs=3, space="PSUM")
                )
                den_psum = ph2.enter_context(
                    tc.tile_pool(name="den_psum", bufs=1, space="PSUM")
                )
                p_pool = ph2.enter_context(tc.tile_pool(name="p", bufs=3))
                nrm_pool = ph2.enter_context(tc.tile_pool(name="nrm", bufs=2))
                aon_pool = ph2.enter_context(tc.tile_pool(name="aon", bufs=4))

                for b in range(8):
                    aops = [
                        ao_psum.tile([128, R], f32, tag="aops", name=f"aops{b}_{i}")
                        for i in range(2)
                    ]
                    denp = den_psum.tile([1, R], f32, tag="denp")
                    ngroups = 2 * (b + 1)
                    nkeys = 2 * ngroups  # 128-key slices in this block

                    def consume(item):
                        k, r0, pt_c = item
                        sl = k % 2
                        first = k == 0
                        last = k == nkeys - 1
                        # AV first so a denominator-bank wait can't block
                        # the AV matmuls behind it in the PE FIFO
                        for dh in range(2):
                            nc.tensor.matmul(
                                aops[dh][:, r0:R],
                                kv_sb[:, D * k + 128 * dh : D * k + 128 * dh + 128],
                                pt_c[:, 512 * sl + r0 : 512 * sl + 512],
                                start=first,
                                stop=last,
                            )
                        nc.tensor.matmul(
                            denp[0:1, r0:R],
                            ones_col[:],
                            pt_c[:, 512 * sl + r0 : 512 * sl + 512],
                            start=first,
                            stop=last,
                            skip_group_check=True,
                        )

                    pending = []

                    def consume_all():
                        for it in pending:
                            consume(it)
                        del pending[:]

                    for kg in range(ngroups):
                        diag = kg >= 2 * b
                        grel = kg - 2 * b
                        st = s_psum.tile([128, 1024], f32, tag="st")
                        pt = p_pool.tile([128, 1024], bf16, tag="pt")
                        if not diag:
                            for sl in range(2):
                                k = 2 * kg + sl
                                for dh in range(2):
                                    nc.tensor.matmul(
                                        st[:, 512 * sl : 512 * sl + 512],
                                        kvT[
                                            :,
                                            S * dh + 128 * k : S * dh + 128 * k + 128,
                                        ],
                                        qT[:, S * dh + R * b : S * dh + R * b + R],
                                        start=(dh == 0),
                                        stop=(dh == 1),
                                    )
                            # one fused ACTIVATE over both key slices
                            nc.scalar.activation(
                                pt[:, 0:1024], st[:, 0:1024], AF.Exp, scale=SCALE
                            )
                            consume_all()
                            pending.append((2 * kg, 0, pt))
                            pending.append((2 * kg + 1, 0, pt))
                        else:
                            for sl in range(2):
                                r0 = 256 * grel + 128 * sl
                                k = 2 * kg + sl
                                for dh in range(2):
                                    nc.tensor.matmul(
                                        st[:, 512 * sl + r0 : 512 * sl + 512],
                                        kvT[
                                            :,
                                            S * dh + 128 * k : S * dh + 128 * k + 128,
                                        ],
                                        qT[
                                            :,
                                            S * dh + R * b + r0 : S * dh + R * b + R,
                                        ],
                                        start=(dh == 0),
                                        stop=(dh == 1),
                                    )
                                nc.vector.tensor_add(
                                    st[:, 512 * sl + r0 : 512 * sl + 512],
                                    st[:, 512 * sl + r0 : 512 * sl + 512],
                                    mask_sb[:, 0 : 512 - r0],
                                )
                                nc.scalar.activation(
                                    pt[:, 512 * sl + r0 : 512 * sl + 512],
                                    st[:, 512 * sl + r0 : 512 * sl + 512],
                                    AF.Exp,
                                    scale=SCALE,
                                )
                                consume_all()
                                pending.append((k, r0, pt))
                    consume_all()
                    # evacuate PSUM fast (frees aops for the next block's
                    # AV matmuls), then normalize in SBUF at 2x DVE rate
                    raws = []
                    for dh in range(2):
                        raw = aon_pool.tile([128, R], bf16, tag="raw")
                        nc.vector.tensor_copy(raw[:], aops[dh][:])
                        raws.append(raw)
                    den_sb = nrm_pool.tile([1, R], bf16, tag="den_sb")
                    with nc.allow_low_precision(
                        reason="1/den in bf16: |rel err| ~0.4% vs 2e-2 budget"
                    ):
                        nc.vector.reciprocal(den_sb[:], denp[0:1, :])
                    bc = nrm_pool.tile([128, R], bf16, tag="bc")
                    nc.gpsimd.partition_broadcast(bc[:], den_sb[:])
                    for dh in range(2):
                        aon = aon_pool.tile([128, R], bf16, tag="aon")
                        nc.vector.tensor_mul(aon[:], raws[dh][:], bc[:])
                        nc.sync.dma_start(
                            ao_send[D * b + 128 * dh : D * b + 128 * dh + 128, :],
                            aon[:],
                        )
                    if b == 3:
                        # prefetch wo only now: keeps the scalar DMA queue
                        # free for the kv/xT tail during the transition
                        nc.scalar.dma_start(wo_sb[:], wo_d[:])
                nc.gpsimd.collective_compute(
                    "AllToAll",
                    mybir.AluOpType.bypass,
                    replica_groups=grp,
                    ins=[ao_send[:]],
                    outs=[ao_recv[:]],
                )

            # ============ phase 3: output projection ============
            with ExitStack() as ph3:
                o_in = ph3.enter_context(tc.tile_pool(name="o_in", bufs=1))
                bor_sb = o_in.tile([1, HID], bf16, tag="bor")
                nc.sync.dma_start(bor_sb[:], bo_row[:])
                bo_bc = o_in.tile([128, HID], bf16, tag="bo_bc")
                nc.gpsimd.partition_broadcast(bo_bc[:], bor_sb[:])
                # 4 aoT tiles so the first out-proj matmuls start after the
                # first quarter of the loads lands
                aoTs = []
                for g in range(4):
                    t = o_in.tile([128, 4 * R], bf16, tag=f"aoT{g}")
                    for kk in range(4):
                        k = 4 * g + kk
                        eng = nc.sync if k % 2 == 0 else nc.scalar
                        eng.dma_start(
                            t[:, R * kk : R * kk + R],
                            ao_recv[128 * k : 128 * k + 128, :],
                        )
                    aoTs.append(t)
                o_psum = ph3.enter_context(
                    tc.tile_pool(name="o_psum", bufs=4, space="PSUM")
                )
                o_out = ph3.enter_context(tc.tile_pool(name="o_out", bufs=2))
                for rc in range(4):
                    osb = o_out.tile([128, HID], bf16, tag="osb")
                    for ncol in range(4):
                        ps = o_psum.tile([128, 512], f32, tag="ops")
                        for k in range(16):
                            nc.tensor.matmul(
                                ps[:],
                                aoTs[k // 4][:, R * (k % 4) + 128 * rc : R * (k % 4) + 128 * rc + 128],
                                wo_sb[
                                    :,
                                    HID * k + 512 * ncol : HID * k + 512 * ncol + 512,
                                ],
                                start=(k == 0),
                                stop=(k == 15),
                            )
                        with nc.allow_low_precision(
                            reason="bf16 output rounding ~0.4% vs 2e-2 budget"
                        ):
                            nc.vector.tensor_add(
                                osb[:, 512 * ncol : 512 * ncol + 512],
                                ps[:],
                                bo_bc[:, 512 * ncol : 512 * ncol + 512],
                            )
                    eng = nc.sync if rc % 2 == 0 else nc.scalar
                    eng.dma_start(out[128 * rc : 128 * rc + 128, :], osb[:])

    nc.compile()
    _BUILT = nc
    return nc


def _make_in_maps(x, wq, bq, wkv, bkv, wo, bo):
    import ml_dtypes

    bfnp = ml_dtypes.bfloat16
    x2d = np.asarray(x, dtype=np.float32).reshape(S, HID)
    # xT_d[c, p, hs*CH+col] = x[CH*c+col, 128*hs+p]
    xT = (
        x2d.reshape(NCH, CH, 16, 128)
        .transpose(0, 3, 2, 1)
        .reshape(NCH, 128, 16 * CH)
        .astype(bfnp)
    )
    wq3 = np.asarray(wq, dtype=np.float32).reshape(HID, NH, D)
    bq2 = np.asarray(bq, dtype=np.float32).reshape(NH, D)
    bkv1 = np.asarray(bkv, dtype=np.float32).reshape(D)
    wkv2 = np.asarray(wkv, dtype=np.float32).reshape(HID, D)
    wo2 = np.asarray(wo, dtype=np.float32).reshape(HID, HID)
    wo_h = (
        wo2.reshape(16, 128, HID).transpose(1, 0, 2).reshape(128, 16 * HID).astype(bfnp)
    )
    shared = {
        "xT": xT,
        "wo2d": wo_h,
        "bo_row": np.asarray(bo, dtype=np.float32).reshape(1, HID).astype(bfnp),
    }
    # kvx[p, hs*512+col] = x[512*j+col, 128*hs+p]
    xr4 = x2d.reshape(NCORES, 512, 16, 128)  # [j, col, hs, p]
    in_maps = []
    for j in range(NCORES):
        m = dict(shared)
        m["kvx"] = (
            np.ascontiguousarray(xr4[j].transpose(2, 1, 0))
            .reshape(128, 16 * 512)
            .astype(bfnp)
        )
        wq_h = wq3[:, j, :]  # [HID, D]
        qk = np.concatenate(
            [wq_h.reshape(16, 128, D), wkv2.reshape(16, 128, D)], axis=2
        )  # [16, 128, 512]
        m["wqkv"] = qk.transpose(1, 0, 2).reshape(128, 16 * 512).astype(bfnp)
        bq_h = bq2[j]
        bqkv = np.stack(
            [bq_h[:128], bq_h[128:], bkv1[:128], bkv1[128:]], axis=1
        )  # [128, 4]
        m["bqkv"] = np.ascontiguousarray(bqkv.astype(np.float32))
        in_maps.append(m)
    return in_maps


def _run(inputs, trace=False, **trace_kwargs):
    from concourse.bass_utils import run_bass_kernel_spmd

    nc = _build()
    in_maps = _make_in_maps(
        inputs["x"],
        inputs["wq"],
        inputs["bq"],
        inputs["wkv"],
        inputs["bkv"],
        inputs["wo"],
        inputs["bo"],
    )
    res = run_bass_kernel_spmd(
        nc, in_maps, list(range(NCORES)), trace=trace, **trace_kwargs
    )
    outs = [np.asarray(res.results[j]["out"]) for j in range(NCORES)]
    full = np.concatenate(outs, axis=0).reshape(1, S, HID).astype(np.float32)
    return full, res


def kernel(**inputs):
    full, _ = _run(inputs, trace=False)
    return full


if __name__ == "__main__":
    rng = np.random.default_rng(0)
    ins = {
        "x": rng.standard_normal((1, S, HID), dtype=np.float32),
        "wq": rng.standard_normal((HID, NH, D), dtype=np.float32) / 45.25,
        "bq": np.zeros((NH, D), np.float32),
        "wkv": rng.standard_normal((HID, 1, D), dtype=np.float32) / 45.25,
        "bkv": np.zeros((1, D), np.float32),
        "wo": rng.standard_normal((NH, D, HID), dtype=np.float32) / 45.25,
        "bo": np.zeros((HID,), np.float32),
        "mask": np.tril(np.ones((S, S), bool))[None, None],
    }
    out = kernel(**ins)
    print("out", out.shape, out.dtype, float(np.abs(out).max()))
